# revision 3
# baseline (speedup 1.0000x reference)
"""Trainium2 Bass kernel for nn_AttentionHead_17042430231165.

out = softmax(min((x@wq.T+qb)@(x@wk.T+kb).T / 256, tri)) @ (x@wv.T+vb)
  x [32, 8192], wk/wq [256, 8192], wv [8192, 8192], tri [32, 32]

Sharding (8 cores):
  - wv rows (VAL) sharded: 1024 rows/core -> each core computes out[:, c*1024:(c+1)*1024]
  - wk/wq rows (KEY) sharded: 32 dims/core -> partial scores s_c = q_c @ k_c.T [32,32]
    AllReduce(sum) of the 4KB partial-score tile gives full scores on every core.
  - x replicated. Softmax computed redundantly per core.

All matmuls contract over IN=8192 -> operands pre-swizzled on host to
[128 partitions, 64 chunks, cols]; x/wk/wq/wv streamed as bf16.

Perf structure (what made this fast vs the 111us first version):
  - x/wkq/bias loads ride the ACT HWDGE ring; the 16MB wv stream rides the
    SP HWDGE ring as 16x1MB DMAs -> score path is not queued behind wv tiles.
  - A prelude 1-byte AllGather (bass kernel-barrier mechanism) is inserted
    right after the engine preambles, so NRT's once-per-NEFF collective
    BARRIER runs from ~7us, hidden under the wv stream, instead of
    serializing before the score AllReduce.
  - Scores kept natural-orientation: one scalar-engine op computes
    e=exp(min(S,tri*256)/256) AND the softmax denominators (accum_out);
    1/256 folded into the activation scale, x256 folded into tri on host.
  - Epilogue: e -> (DVE transpose) eT -> two f32r matmuls (a@v) -> fused
    (pu*rz)+vb scalar_tensor_tensor -> HWDGE store. Only min/exp/transpose
    depend on the AllReduce and they complete under the stream.
"""
import sys

for _p in (
    "/root/.axon_site",
    "/root/.axon_site/_ro/trn_rl_repo",
    "/root/.axon_site/_ro/pypackages",
):
    if _p not in sys.path:
        sys.path.insert(0, _p)

import numpy as np
from ml_dtypes import bfloat16

from concourse import bacc, tile
from concourse import mybir
from concourse.bass_utils import run_bass_kernel_spmd

W = 32          # window (seq) size
IN = 8192       # in_size
KEY = 256       # key_size
VAL = 8192      # value_size
P = 128         # SBUF partitions
NCH = IN // P   # 64 contraction chunks
NCORES = 8
KSH = KEY // NCORES   # 32 key dims per core
VSH = VAL // NCORES   # 1024 value dims per core
SCALE = 1.0 / 256.0
NDMA = 16       # wv stream DMA count (1MB each)
CCD = NCH // NDMA  # contraction chunks per streamed wv DMA tile
NT = 512        # moving free dim per matmul (fp32 max)

F32 = mybir.dt.float32
F32R = mybir.dt.float32r
BF16 = mybir.dt.bfloat16

_NC = None


def _build():
    global _NC
    if _NC is not None:
        return _NC
    nc = bacc.Bacc(None, target_bir_lowering=False, debug=False, num_devices=NCORES)

    X = nc.declare_dram_parameter("X", [P, NCH, W], BF16, isOutput=False)
    WKQ = nc.declare_dram_parameter("WKQ", [P, NCH, 2 * KSH], BF16, isOutput=False)
    WV = nc.declare_dram_parameter("WV", [P, NCH, VSH], BF16, isOutput=False)
    KQB = nc.declare_dram_parameter("KQB", [2 * KSH, 1], F32, isOutput=False)
    VB = nc.declare_dram_parameter("VB", [W, VSH], F32, isOutput=False)
    TRI = nc.declare_dram_parameter("TRI", [W, W], F32, isOutput=False)
    OUT = nc.declare_dram_parameter("out", [W, VSH], F32, isOutput=True)

    cc_in = nc.dram_tensor("cc_in", [W, W], F32)
    cc_out = nc.dram_tensor("cc_out", [W, W], F32, addr_space="Shared")

    with tile.TileContext(nc) as tc:
        with (
            tc.tile_pool(name="const", bufs=1) as cpool,
            tc.tile_pool(name="wv", bufs=6) as wpool,
            tc.tile_pool(name="small", bufs=1) as spool,
            tc.tile_pool(name="psum", bufs=1, space="PSUM") as ppool,
        ):
            # constants / replicated inputs -- all on the ACT HWDGE ring so
            # they are not queued behind the wv stream (SP ring).
            x_sb = cpool.tile([P, NCH, W], BF16)
            nc.scalar.dma_start(out=x_sb[:], in_=X[:])
            wkq_sb = cpool.tile([P, NCH, 2 * KSH], BF16)
            nc.scalar.dma_start(out=wkq_sb[:], in_=WKQ[:])
            kqb_sb = cpool.tile([2 * KSH, 1], F32)
            nc.scalar.dma_start(out=kqb_sb[:], in_=KQB[:])
            tri_sb = cpool.tile([W, W], F32)
            nc.scalar.dma_start(out=tri_sb[:], in_=TRI[:])
            vb_sb = cpool.tile([W, VSH], F32)
            nc.scalar.dma_start(out=vb_sb[:], in_=VB[:])

            # kqT = [wk; wq] @ x.T + [kb; qb]  [2*KSH, W] in one chain
            pkq = ppool.tile([2 * KSH, W], F32)
            for c in range(NCH):
                nc.tensor.matmul(
                    pkq[:], wkq_sb[:, c, :], x_sb[:, c, :],
                    start=(c == 0), stop=(c == NCH - 1),
                )
            kqT = spool.tile([2 * KSH, W], F32)
            nc.vector.tensor_scalar_add(kqT[:], pkq[:], kqb_sb[:])
            # q half copied to a base-0 tile (matmul needs equal base partitions)
            qT = spool.tile([KSH, W], F32)
            nc.scalar.dma_start(out=qT[:], in_=kqT[KSH:2 * KSH, :])

            # partial scores s[m,n] = sum_d q[m,d] k[n,d] (this core's d-slice)
            ps = ppool.tile([W, W], F32)
            nc.tensor.matmul(ps[:], qT[:], kqT[0:KSH, :])
            s_sb = spool.tile([W, W], F32)
            nc.vector.tensor_copy(s_sb[:], ps[:])

            # AllReduce the partial scores across 8 cores (4KB)
            nc.scalar.dma_start(out=cc_in[:], in_=s_sb[:])
            nc.gpsimd.collective_compute(
                "AllReduce",
                mybir.AluOpType.add,
                replica_groups=[list(range(NCORES))],
                ins=[cc_in.ap().opt()],
                outs=[cc_out.ap().opt()],
            )
            S_sb = spool.tile([W, W], F32)
            nc.scalar.dma_start(out=S_sb[:], in_=cc_out[:])

            # v = x @ wv_c.T streamed over 64 contraction chunks (SP ring)
            pv0 = ppool.tile([W, NT], F32)
            pv1 = ppool.tile([W, NT], F32)
            for d in range(NDMA):
                wt = wpool.tile([P, CCD, VSH], BF16, tag="wvstream")
                nc.sync.dma_start(out=wt[:], in_=WV[:, d * CCD:(d + 1) * CCD, :])
                for i in range(CCD):
                    c = d * CCD + i
                    nc.tensor.matmul(
                        pv0[:], x_sb[:, c, :], wt[:, i, 0:NT],
                        start=(c == 0), stop=(c == NCH - 1),
                    )
                    nc.tensor.matmul(
                        pv1[:], x_sb[:, c, :], wt[:, i, NT:VSH],
                        start=(c == 0), stop=(c == NCH - 1),
                    )

            # AR-dependent prologue of the epilogue (runs under the stream):
            # e = exp(min(S, tri*256)/256), Z row-sums via accum_out, eT.
            m_sb = spool.tile([W, W], F32)
            nc.vector.tensor_tensor(m_sb[:], S_sb[:], tri_sb[:], mybir.AluOpType.min)
            e_sb = spool.tile([W, W], F32)
            pz = spool.tile([W, 1], F32)
            nc.scalar.activation(
                e_sb[:], m_sb[:], mybir.ActivationFunctionType.Exp,
                scale=SCALE, accum_out=pz[:],
            )
            rz = spool.tile([W, 1], F32)
            nc.vector.reciprocal(rz[:], pz[:])
            eTf = spool.tile([W, W], F32)
            nc.vector.transpose(eTf[:], e_sb[:])
            eT = spool.tile([W, W], F32R)
            nc.vector.tensor_copy(eT[:], eTf[:])

            # tail: v copies, a@v, fused (pu*rz)+vb, store
            for j, pv in enumerate((pv0, pv1)):
                v_sb = spool.tile([W, NT], F32R, tag=f"v{j}")
                nc.vector.tensor_copy(v_sb[:], pv[:])
                pu = ppool.tile([W, NT], F32, tag=f"pu{j}")
                nc.tensor.matmul(pu[:], eT[:], v_sb[:])
                o_sb = spool.tile([W, NT], F32, tag=f"o{j}")
                nc.vector.scalar_tensor_tensor(
                    o_sb[:], pu[:], rz[:], vb_sb[:, j * NT:(j + 1) * NT],
                    mybir.AluOpType.mult, mybir.AluOpType.add,
                )
                nc.sync.dma_start(out=OUT[:, j * NT:(j + 1) * NT], in_=o_sb[:])

    # Prelude 1-byte AllGather right after the engine preambles: NRT's
    # once-per-NEFF collective BARRIER attaches to it at ~7us (hidden under
    # the wv stream) instead of serializing before the score AllReduce.
    nc._bir_kernel_barrier_sem_replica_groups.extend([set(range(NCORES))])

    nc.compile()
    _NC = nc
    return nc


def _swizzle(mat_t):
    """[rows=IN, cols] (transposed so IN is dim 0) -> bf16 [P, NCH, cols]."""
    rows, cols = mat_t.shape
    assert rows == IN
    return np.ascontiguousarray(
        mat_t.reshape(NCH, P, cols).transpose(1, 0, 2).astype(bfloat16))


def _make_in_maps(x, wk_w, wk_b, wq_w, wq_b, wv_w, wv_b, tri):
    x = np.asarray(x, dtype=np.float32)
    X_dev = _swizzle(np.ascontiguousarray(x.T))
    TRI = np.ascontiguousarray(np.asarray(tri, dtype=np.float32) * 256.0)
    in_maps = []
    for c in range(NCORES):
        wk_sh = np.asarray(wk_w[c * KSH:(c + 1) * KSH, :], dtype=np.float32)
        wq_sh = np.asarray(wq_w[c * KSH:(c + 1) * KSH, :], dtype=np.float32)
        wv_sh = np.asarray(wv_w[c * VSH:(c + 1) * VSH, :], dtype=np.float32)
        wv_sw = _swizzle(np.ascontiguousarray(wv_sh.T))
        in_maps.append({
            "X": X_dev,
            "WKQ": _swizzle(np.ascontiguousarray(
                np.concatenate([wk_sh, wq_sh], axis=0).T)),
            "WV": wv_sw,
            "KQB": np.ascontiguousarray(np.concatenate([
                np.asarray(wk_b[c * KSH:(c + 1) * KSH], dtype=np.float32),
                np.asarray(wq_b[c * KSH:(c + 1) * KSH], dtype=np.float32),
            ]).reshape(2 * KSH, 1)),
            "VB": np.ascontiguousarray(np.broadcast_to(
                np.asarray(wv_b[c * VSH:(c + 1) * VSH], dtype=np.float32).reshape(1, VSH),
                (W, VSH))),
            "TRI": TRI,
        })
    return in_maps


def run(inputs, trace=False):
    """Build + run on 8 cores; returns (full_output, BassKernelResults)."""
    nc = _build()
    in_maps = _make_in_maps(**inputs)
    res = run_bass_kernel_spmd(
        nc, in_maps, core_ids=list(range(NCORES)), trace=trace,
    )
    out = np.concatenate([res.results[c]["out"] for c in range(NCORES)], axis=1)
    return out, res


def kernel(**inputs):
    out, _ = run(inputs, trace=False)
    return out


if __name__ == "__main__":
    rng = np.random.default_rng(0)
    ins = {
        "x": rng.standard_normal((W, IN), dtype=np.float32),
        "wk_w": rng.standard_normal((KEY, IN), dtype=np.float32) / 90.5,
        "wk_b": rng.standard_normal((KEY,), dtype=np.float32) / 90.5,
        "wq_w": rng.standard_normal((KEY, IN), dtype=np.float32) / 90.5,
        "wq_b": rng.standard_normal((KEY,), dtype=np.float32) / 90.5,
        "wv_w": rng.standard_normal((VAL, IN), dtype=np.float32) / 90.5,
        "wv_b": rng.standard_normal((VAL,), dtype=np.float32) / 90.5,
        "tri": ((np.tril(np.full((W, W), 2.0, dtype=np.float32)) - 1.0) * 1e5),
    }
    out = kernel(**ins)
    print("out", out.shape, out.dtype, np.abs(out).mean())


# revision 12
# speedup vs baseline: 1.2886x; 1.2886x over previous
"""Trainium2 Bass kernel for nn_AttentionHead_17042430231165.

out = softmax(min((x@wq.T+qb)@(x@wk.T+kb).T / 256, tri)) @ (x@wv.T+vb)
  x [32, 8192], wk/wq [256, 8192], wv [8192, 8192], tri [32, 32]

Sharding (8 cores):
  - wv rows (VAL) sharded: 1024 rows/core -> each core computes out[:, c*1024:(c+1)*1024]
  - wk/wq fully REPLICATED on every core; scores and softmax computed
    locally. No collectives: profiling showed NRT's CC stream has
    ~60-85us of intrinsic bootstrap latency on this runtime (a 1-byte
    AllGather triggered at 7.9us completed at 83.6us), so the extra
    8.4MB/core of wkq HBM traffic (~25us) is far cheaper than any
    cross-core reduction of the 4KB partial scores.

Structure:
  - One HWDGE stream on the SP ring: x (0.5MB) -> wkq (8.4MB, 8x1MB)
    -> wv (16MB, 16x1MB), all bf16, [128 part, chunk, cols] swizzled.
  - PE: kq chain (x stationary, wkq moving 512 cols) runs while wv
    streams in behind it; then 128 v-matmuls chase the wv tiles.
  - Scores locally: biases folded in via a 1-partition matmul into the
    same PSUM group; kq [32,512] -> DVE 32x32 block transpose -> 8
    accumulating [32dims x 32win] matmuls -> full scores.
  - Epilogue: one scalar-engine op computes e=exp(min(S,tri*256)/256)
    AND the softmax denominators (accum_out); x256 folded into tri on
    host. e -> DVE transpose -> f32r -> two a@v matmuls -> fused
    (pu*rz)+vb -> HWDGE store. All score work is hidden under the wv
    stream; the post-stream tail is ~2us.
"""
import sys

for _p in (
    "/root/.axon_site",
    "/root/.axon_site/_ro/trn_rl_repo",
    "/root/.axon_site/_ro/pypackages",
):
    if _p not in sys.path:
        sys.path.insert(0, _p)

import numpy as np
from ml_dtypes import bfloat16

from concourse import bacc, tile
from concourse import mybir
from concourse.bass_utils import run_bass_kernel_spmd

W = 32          # window (seq) size
IN = 8192       # in_size
KEY = 256       # key_size
VAL = 8192      # value_size
P = 128         # SBUF partitions
NCH = IN // P   # 64 contraction chunks
NCORES = 8
VSH = VAL // NCORES   # 1024 value dims per core
KQ = 2 * KEY    # 512 = full [k | q] projection width, replicated
SCALE = 1.0 / 256.0
NKQD = 4        # wkq stream DMA count (1MB each fp8, 16 chunks per tile)
KQC = NCH // NKQD
NVD = 16        # wv stream DMA count (1MB each, 4 chunks per tile)
VC = NCH // NVD
NT = 512        # moving free dim per matmul (fp32 max)

F32 = mybir.dt.float32
F32R = mybir.dt.float32r
BF16 = mybir.dt.bfloat16
F8 = mybir.dt.float8e4

_NC = None


def _build():
    global _NC
    if _NC is not None:
        return _NC
    nc = bacc.Bacc(None, target_bir_lowering=False, debug=False, num_devices=NCORES)

    X = nc.declare_dram_parameter("X", [P, NCH, W], BF16, isOutput=False)
    X8 = nc.declare_dram_parameter("X8", [P, NCH, W], F8, isOutput=False)
    WKQ = nc.declare_dram_parameter("WKQ", [P, NCH, KQ], F8, isOutput=False)
    WV = nc.declare_dram_parameter("WV", [P, NCH, VSH], BF16, isOutput=False)
    KQB = nc.declare_dram_parameter("KQB", [1, KQ], F8, isOutput=False)
    VB = nc.declare_dram_parameter("VB", [W, VSH], F32, isOutput=False)
    TRI = nc.declare_dram_parameter("TRI", [W, W], F32, isOutput=False)
    SCL = nc.declare_dram_parameter("SCL", [W, 1], F32, isOutput=False)
    OUT = nc.declare_dram_parameter("out", [W, VSH], F32, isOutput=True)

    with tile.TileContext(nc) as tc:
        with (
            tc.tile_pool(name="const", bufs=1) as cpool,
            tc.tile_pool(name="kq", bufs=4) as kpool,
            tc.tile_pool(name="wv", bufs=6) as wpool,
            tc.tile_pool(name="small", bufs=1) as spool,
            tc.tile_pool(name="psum", bufs=1, space="PSUM") as ppool,
        ):
            # x8 leads the SP stream so the fp8 kq chain can start ASAP;
            # small constants ride the otherwise-idle ACT ring.
            x8_sb = cpool.tile([P, NCH, W], F8)
            nc.sync.dma_start(out=x8_sb[:], in_=X8[:])
            kqb_sb = cpool.tile([1, KQ], F8)
            nc.scalar.dma_start(out=kqb_sb[:], in_=KQB[:])
            tri_sb = cpool.tile([W, W], F32)
            nc.scalar.dma_start(out=tri_sb[:], in_=TRI[:])
            scl_sb = cpool.tile([W, 1], F32)
            nc.scalar.dma_start(out=scl_sb[:], in_=SCL[:])
            vb_sb = cpool.tile([W, VSH], F32)
            nc.scalar.dma_start(out=vb_sb[:], in_=VB[:])
            ones1 = cpool.tile([1, W], F8)
            nc.vector.memset(ones1[:], 1.0)

            # kq' = S*(x @ [wk; wq].T + 1*[kb; qb])  -> [32, 512] natural rows
            pkq = ppool.tile([W, KQ], F32)
            for d in range(NKQD):
                kt = kpool.tile([P, KQC, KQ], F8, tag="kqstream")
                nc.sync.dma_start(out=kt[:], in_=WKQ[:, d * KQC:(d + 1) * KQC, :])
                for i in range(KQC):
                    c = d * KQC + i
                    nc.tensor.matmul(
                        pkq[:], x8_sb[:, c, :], kt[:, i, :],
                        start=(c == 0), stop=False,
                    )
            # bias via 1-partition rank-1 matmul into the same PSUM group
            nc.tensor.matmul(pkq[:], ones1[:], kqb_sb[:], start=False, stop=True,
                             skip_group_check=True)
            kq_sb = spool.tile([W, KQ], F32)
            nc.vector.tensor_copy(kq_sb[:], pkq[:])
            # 32x32 block transpose: block b holds kq[:, 32b:32b+32].T
            kqt = spool.tile([W, KQ], F32)
            nc.vector.transpose(kqt[:], kq_sb[:])
            # scores s[m,n] = sum_g q_g[:,m].T @ k_g[:,n], k blocks 0..7, q blocks 8..15
            ps = ppool.tile([W, W], F32)
            for g in range(8):
                nc.tensor.matmul(
                    ps[:], kqt[:, KEY + g * W:KEY + (g + 1) * W], kqt[:, g * W:(g + 1) * W],
                    start=(g == 0), stop=(g == 7),
                )
            S_sb = spool.tile([W, W], F32)
            nc.vector.tensor_copy(S_sb[:], ps[:])

            # bf16 x for the v chain, queued behind the wkq stream
            x_sb = cpool.tile([P, NCH, W], BF16)
            nc.sync.dma_start(out=x_sb[:], in_=X[:])

            # v = x @ wv_c.T streamed over 64 contraction chunks (SP ring)
            pv0 = ppool.tile([W, NT], F32)
            pv1 = ppool.tile([W, NT], F32)
            for d in range(NVD):
                wt = wpool.tile([P, VC, VSH], BF16, tag="wvstream")
                nc.sync.dma_start(out=wt[:], in_=WV[:, d * VC:(d + 1) * VC, :])
                for i in range(VC):
                    c = d * VC + i
                    nc.tensor.matmul(
                        pv0[:], x_sb[:, c, :], wt[:, i, 0:NT],
                        start=(c == 0), stop=(c == NCH - 1),
                    )
                    nc.tensor.matmul(
                        pv1[:], x_sb[:, c, :], wt[:, i, NT:VSH],
                        start=(c == 0), stop=(c == NCH - 1),
                    )

            # softmax numerators + denominators (hidden under the stream):
            # e = exp(min(S, tri*256)/256), Z row-sums via accum_out, then eT.
            m_sb = spool.tile([W, W], F32)
            nc.vector.tensor_tensor(m_sb[:], S_sb[:], tri_sb[:], mybir.AluOpType.min)
            e_sb = spool.tile([W, W], F32)
            pz = spool.tile([W, 1], F32)
            nc.scalar.activation(
                e_sb[:], m_sb[:], mybir.ActivationFunctionType.Exp,
                scale=scl_sb[:], accum_out=pz[:],
            )
            rz = spool.tile([W, 1], F32)
            nc.vector.reciprocal(rz[:], pz[:])
            eTf = spool.tile([W, W], F32)
            nc.vector.transpose(eTf[:], e_sb[:])
            eT = spool.tile([W, W], F32R)
            nc.vector.tensor_copy(eT[:], eTf[:])

            # tail: v copies, a@v, fused (pu*rz)+vb, one merged store
            o_sb = spool.tile([W, VSH], F32)
            for j, pv in enumerate((pv0, pv1)):
                v_sb = spool.tile([W, NT], F32R, tag=f"v{j}")
                nc.vector.tensor_copy(v_sb[:], pv[:])
                pu = ppool.tile([W, NT], F32, tag=f"pu{j}")
                nc.tensor.matmul(pu[:], eT[:], v_sb[:])
                nc.vector.scalar_tensor_tensor(
                    o_sb[:, j * NT:(j + 1) * NT], pu[:], rz[:],
                    vb_sb[:, j * NT:(j + 1) * NT],
                    mybir.AluOpType.mult, mybir.AluOpType.add,
                )
            nc.scalar.dma_start(out=OUT[:], in_=o_sb[:])

    nc.compile()
    _NC = nc
    return nc


def _swizzle(mat_t):
    """[rows=IN, cols] (transposed so IN is dim 0) -> bf16 [P, NCH, cols]."""
    rows, cols = mat_t.shape
    assert rows == IN
    return np.ascontiguousarray(
        mat_t.reshape(NCH, P, cols).transpose(1, 0, 2).astype(bfloat16))


def _swizzle8(mat_t):
    """[rows=IN, cols] -> fp8 e4m3 [P, NCH, cols] (clip to TRN max +-240)."""
    rows, cols = mat_t.shape
    assert rows == IN
    from ml_dtypes import float8_e4m3
    return np.ascontiguousarray(
        np.clip(mat_t, -240.0, 240.0)
        .reshape(NCH, P, cols).transpose(1, 0, 2).astype(float8_e4m3))


def _make_in_maps(x, wk_w, wk_b, wq_w, wq_b, wv_w, wv_b, tri):
    from ml_dtypes import float8_e4m3
    x = np.asarray(x, dtype=np.float32)
    xT = np.ascontiguousarray(x.T)
    X_dev = _swizzle(xT)
    X8_dev = _swizzle8(xT)
    # fp8 kq path: scale wk/wq (and biases) by S so values sit ~N(0,1) in
    # e4m3; scores come out as S^2 * raw and 1/(256*S^2) is applied inside
    # the exp (runtime scale tensor), with tri pre-scaled to match.
    wkq = np.concatenate([np.asarray(wk_w, dtype=np.float32),
                          np.asarray(wq_w, dtype=np.float32)], axis=0)
    S = 1.0 / max(float(np.std(wkq)), 1e-12)
    TRI = np.ascontiguousarray(
        np.asarray(tri, dtype=np.float32) * (256.0 * S * S))
    SCL = np.full((W, 1), SCALE / (S * S), dtype=np.float32)
    WKQ_dev = _swizzle8(np.ascontiguousarray(wkq.T) * S)
    KQB_dev = np.ascontiguousarray(np.clip(np.concatenate([
        np.asarray(wk_b, dtype=np.float32),
        np.asarray(wq_b, dtype=np.float32),
    ]) * S, -240.0, 240.0).reshape(1, KQ).astype(float8_e4m3))
    in_maps = []
    for c in range(NCORES):
        wv_sh = np.asarray(wv_w[c * VSH:(c + 1) * VSH, :], dtype=np.float32)
        in_maps.append({
            "X": X_dev,
            "X8": X8_dev,
            "WKQ": WKQ_dev,
            "WV": _swizzle(np.ascontiguousarray(wv_sh.T)),
            "KQB": KQB_dev,
            "VB": np.ascontiguousarray(np.broadcast_to(
                np.asarray(wv_b[c * VSH:(c + 1) * VSH], dtype=np.float32).reshape(1, VSH),
                (W, VSH))),
            "TRI": TRI,
            "SCL": SCL,
        })
    return in_maps


def run(inputs, trace=False):
    """Build + run on 8 cores; returns (full_output, BassKernelResults)."""
    nc = _build()
    in_maps = _make_in_maps(**inputs)
    res = run_bass_kernel_spmd(
        nc, in_maps, core_ids=list(range(NCORES)), trace=trace,
    )
    out = np.concatenate([res.results[c]["out"] for c in range(NCORES)], axis=1)
    return out, res


def kernel(**inputs):
    out, _ = run(inputs, trace=False)
    return out


if __name__ == "__main__":
    rng = np.random.default_rng(0)
    ins = {
        "x": rng.standard_normal((W, IN), dtype=np.float32),
        "wk_w": rng.standard_normal((KEY, IN), dtype=np.float32) / 90.5,
        "wk_b": rng.standard_normal((KEY,), dtype=np.float32) / 90.5,
        "wq_w": rng.standard_normal((KEY, IN), dtype=np.float32) / 90.5,
        "wq_b": rng.standard_normal((KEY,), dtype=np.float32) / 90.5,
        "wv_w": rng.standard_normal((VAL, IN), dtype=np.float32) / 90.5,
        "wv_b": rng.standard_normal((VAL,), dtype=np.float32) / 90.5,
        "tri": ((np.tril(np.full((W, W), 2.0, dtype=np.float32)) - 1.0) * 1e5),
    }
    out = kernel(**ins)
    print("out", out.shape, out.dtype, np.abs(out).mean())


# revision 15
# speedup vs baseline: 1.4540x; 1.1283x over previous
"""Trainium2 Bass kernel for nn_AttentionHead_17042430231165.

out = softmax(min((x@wq.T+qb)@(x@wk.T+kb).T / 256, tri)) @ (x@wv.T+vb)
  x [32, 8192], wk/wq [256, 8192], wv [8192, 8192], tri [32, 32]

Sharding (8 cores):
  - wv rows (VAL) sharded: 1024 rows/core -> each core computes out[:, c*1024:(c+1)*1024]
  - wk/wq fully REPLICATED on every core; scores and softmax computed
    locally. No collectives: profiling showed NRT's CC stream has
    ~60-85us of intrinsic bootstrap latency on this runtime (a 1-byte
    AllGather triggered at 7.9us completed at 83.6us), so the extra
    8.4MB/core of wkq HBM traffic (~25us) is far cheaper than any
    cross-core reduction of the 4KB partial scores.

Structure:
  - One HWDGE stream on the SP ring: x (0.5MB) -> wkq (8.4MB, 8x1MB)
    -> wv (16MB, 16x1MB), all bf16, [128 part, chunk, cols] swizzled.
  - PE: kq chain (x stationary, wkq moving 512 cols) runs while wv
    streams in behind it; then 128 v-matmuls chase the wv tiles.
  - Scores locally: biases folded in via a 1-partition matmul into the
    same PSUM group; kq [32,512] -> DVE 32x32 block transpose -> 8
    accumulating [32dims x 32win] matmuls -> full scores.
  - Epilogue: one scalar-engine op computes e=exp(min(S,tri*256)/256)
    AND the softmax denominators (accum_out); x256 folded into tri on
    host. e -> DVE transpose -> f32r -> two a@v matmuls -> fused
    (pu*rz)+vb -> HWDGE store. All score work is hidden under the wv
    stream; the post-stream tail is ~2us.
"""
import sys

for _p in (
    "/root/.axon_site",
    "/root/.axon_site/_ro/trn_rl_repo",
    "/root/.axon_site/_ro/pypackages",
):
    if _p not in sys.path:
        sys.path.insert(0, _p)

import numpy as np
from ml_dtypes import bfloat16

from concourse import bacc, tile
from concourse import mybir
from concourse.bass_utils import run_bass_kernel_spmd

W = 32          # window (seq) size
IN = 8192       # in_size
KEY = 256       # key_size
VAL = 8192      # value_size
P = 128         # SBUF partitions
NCH = IN // P   # 64 contraction chunks
NCORES = 8
VSH = VAL // NCORES   # 1024 value dims per core
KQ = 2 * KEY    # 512 = full [k | q] projection width, replicated
SCALE = 1.0 / 256.0
NKQD = 4        # wkq stream DMA count (1MB each fp8, 16 chunks per tile)
KQC = NCH // NKQD
NVD = 32        # wv stream DMA count (0.5MB each, 2 chunks per tile)
VC = NCH // NVD
NT = 512        # moving free dim per matmul (fp32 max)

F32 = mybir.dt.float32
F32R = mybir.dt.float32r
BF16 = mybir.dt.bfloat16
F8 = mybir.dt.float8e4

_NC = None


def _build():
    global _NC
    if _NC is not None:
        return _NC
    nc = bacc.Bacc(None, target_bir_lowering=False, debug=False, num_devices=NCORES)

    X = nc.declare_dram_parameter("X", [P, NCH, W], BF16, isOutput=False)
    X8 = nc.declare_dram_parameter("X8", [P, NCH, W], F8, isOutput=False)
    WKQ = nc.declare_dram_parameter("WKQ", [P, NCH, KQ], F8, isOutput=False)
    WV = nc.declare_dram_parameter("WV", [P, NCH, VSH], BF16, isOutput=False)
    KQB = nc.declare_dram_parameter("KQB", [1, KQ], F8, isOutput=False)
    VB = nc.declare_dram_parameter("VB", [W, VSH], F32, isOutput=False)
    TRI = nc.declare_dram_parameter("TRI", [W, W], F32, isOutput=False)
    SCL = nc.declare_dram_parameter("SCL", [W, 1], F32, isOutput=False)
    OUT = nc.declare_dram_parameter("out", [W, VSH], F32, isOutput=True)

    with tile.TileContext(nc) as tc:
        with (
            tc.tile_pool(name="const", bufs=1) as cpool,
            tc.tile_pool(name="kq", bufs=4) as kpool,
            tc.tile_pool(name="wv", bufs=10) as wpool,
            tc.tile_pool(name="small", bufs=1) as spool,
            tc.tile_pool(name="psum", bufs=1, space="PSUM") as ppool,
        ):
            # x8 leads the SP stream so the fp8 kq chain can start ASAP;
            # small constants ride the otherwise-idle ACT ring.
            x8_sb = cpool.tile([P, NCH, W], F8)
            nc.sync.dma_start(out=x8_sb[:], in_=X8[:])
            kqb_sb = cpool.tile([1, KQ], F8)
            nc.scalar.dma_start(out=kqb_sb[:], in_=KQB[:])
            tri_sb = cpool.tile([W, W], F32)
            nc.scalar.dma_start(out=tri_sb[:], in_=TRI[:])
            scl_sb = cpool.tile([W, 1], F32)
            nc.scalar.dma_start(out=scl_sb[:], in_=SCL[:])
            vb_sb = cpool.tile([W, VSH], F32)
            nc.scalar.dma_start(out=vb_sb[:], in_=VB[:])
            ones1 = cpool.tile([1, W], F8)
            nc.vector.memset(ones1[:], 1.0)

            # kq' = S*(x @ [wk; wq].T + 1*[kb; qb])  -> [32, 512] natural rows
            pkq = ppool.tile([W, KQ], F32)
            for d in range(NKQD):
                kt = kpool.tile([P, KQC, KQ], F8, tag="kqstream")
                nc.sync.dma_start(out=kt[:], in_=WKQ[:, d * KQC:(d + 1) * KQC, :])
                for i in range(KQC):
                    c = d * KQC + i
                    nc.tensor.matmul(
                        pkq[:], x8_sb[:, c, :], kt[:, i, :],
                        start=(c == 0), stop=False,
                    )
            # bias via 1-partition rank-1 matmul into the same PSUM group
            nc.tensor.matmul(pkq[:], ones1[:], kqb_sb[:], start=False, stop=True,
                             skip_group_check=True)
            kq_sb = spool.tile([W, KQ], F32)
            nc.vector.tensor_copy(kq_sb[:], pkq[:])
            # 32x32 block transpose: block b holds kq[:, 32b:32b+32].T
            kqt = spool.tile([W, KQ], F32)
            nc.vector.transpose(kqt[:], kq_sb[:])
            # scores s[m,n] = sum_g q_g[:,m].T @ k_g[:,n], k blocks 0..7, q blocks 8..15
            ps = ppool.tile([W, W], F32)
            for g in range(8):
                nc.tensor.matmul(
                    ps[:], kqt[:, KEY + g * W:KEY + (g + 1) * W], kqt[:, g * W:(g + 1) * W],
                    start=(g == 0), stop=(g == 7),
                )
            S_sb = spool.tile([W, W], F32)
            nc.vector.tensor_copy(S_sb[:], ps[:])

            # bf16 x for the v chain, queued behind the wkq stream
            x_sb = cpool.tile([P, NCH, W], BF16)
            nc.sync.dma_start(out=x_sb[:], in_=X[:])

            # v = x @ wv_c.T streamed over 64 contraction chunks (SP ring)
            pv0 = ppool.tile([W, NT], F32)
            pv1 = ppool.tile([W, NT], F32)
            for d in range(NVD):
                wt = wpool.tile([P, VC, VSH], BF16, tag="wvstream")
                nc.sync.dma_start(out=wt[:], in_=WV[:, d * VC:(d + 1) * VC, :])
                for i in range(VC):
                    c = d * VC + i
                    nc.tensor.matmul(
                        pv0[:], x_sb[:, c, :], wt[:, i, 0:NT],
                        start=(c == 0), stop=(c == NCH - 1),
                    )
                    nc.tensor.matmul(
                        pv1[:], x_sb[:, c, :], wt[:, i, NT:VSH],
                        start=(c == 0), stop=(c == NCH - 1),
                    )

            # softmax numerators + denominators (hidden under the stream):
            # e = exp(min(S, tri*256)/256), Z row-sums via accum_out, then eT.
            m_sb = spool.tile([W, W], F32)
            nc.vector.tensor_tensor(m_sb[:], S_sb[:], tri_sb[:], mybir.AluOpType.min)
            e_sb = spool.tile([W, W], F32)
            pz = spool.tile([W, 1], F32)
            nc.scalar.activation(
                e_sb[:], m_sb[:], mybir.ActivationFunctionType.Exp,
                scale=scl_sb[:], accum_out=pz[:],
            )
            rz = spool.tile([W, 1], F32)
            nc.vector.reciprocal(rz[:], pz[:])
            eTf = spool.tile([W, W], F32)
            nc.vector.transpose(eTf[:], e_sb[:])
            eT = spool.tile([W, W], F32R)
            nc.vector.tensor_copy(eT[:], eTf[:])

            # tail: v copies, a@v, fused (pu*rz)+vb, stores overlapped on
            # the two HWDGE rings
            for j, pv in enumerate((pv0, pv1)):
                v_sb = spool.tile([W, NT], F32R, tag=f"v{j}")
                nc.vector.tensor_copy(v_sb[:], pv[:])
                pu = ppool.tile([W, NT], F32, tag=f"pu{j}")
                nc.tensor.matmul(pu[:], eT[:], v_sb[:])
                o_sb = spool.tile([W, NT], F32, tag=f"o{j}")
                nc.vector.scalar_tensor_tensor(
                    o_sb[:], pu[:], rz[:], vb_sb[:, j * NT:(j + 1) * NT],
                    mybir.AluOpType.mult, mybir.AluOpType.add,
                )
                eng = nc.scalar if j == 0 else nc.sync
                eng.dma_start(out=OUT[:, j * NT:(j + 1) * NT], in_=o_sb[:])

    nc.compile()
    _NC = nc
    return nc


def _swizzle(mat_t):
    """[rows=IN, cols] (transposed so IN is dim 0) -> bf16 [P, NCH, cols]."""
    rows, cols = mat_t.shape
    assert rows == IN
    return np.ascontiguousarray(
        mat_t.reshape(NCH, P, cols).transpose(1, 0, 2).astype(bfloat16))


def _swizzle8(mat_t):
    """[rows=IN, cols] -> fp8 e4m3 [P, NCH, cols] (clip to TRN max +-240)."""
    rows, cols = mat_t.shape
    assert rows == IN
    from ml_dtypes import float8_e4m3
    return np.ascontiguousarray(
        np.clip(mat_t, -240.0, 240.0)
        .reshape(NCH, P, cols).transpose(1, 0, 2).astype(float8_e4m3))


def _make_in_maps(x, wk_w, wk_b, wq_w, wq_b, wv_w, wv_b, tri):
    from ml_dtypes import float8_e4m3
    x = np.asarray(x, dtype=np.float32)
    xT = np.ascontiguousarray(x.T)
    X_dev = _swizzle(xT)
    X8_dev = _swizzle8(xT)
    # fp8 kq path: scale wk/wq (and biases) by S so values sit ~N(0,1) in
    # e4m3; scores come out as S^2 * raw and 1/(256*S^2) is applied inside
    # the exp (runtime scale tensor), with tri pre-scaled to match.
    wkq = np.concatenate([np.asarray(wk_w, dtype=np.float32),
                          np.asarray(wq_w, dtype=np.float32)], axis=0)
    S = 1.0 / max(float(np.std(wkq)), 1e-12)
    TRI = np.ascontiguousarray(
        np.asarray(tri, dtype=np.float32) * (256.0 * S * S))
    SCL = np.full((W, 1), SCALE / (S * S), dtype=np.float32)
    WKQ_dev = _swizzle8(np.ascontiguousarray(wkq.T) * S)
    KQB_dev = np.ascontiguousarray(np.clip(np.concatenate([
        np.asarray(wk_b, dtype=np.float32),
        np.asarray(wq_b, dtype=np.float32),
    ]) * S, -240.0, 240.0).reshape(1, KQ).astype(float8_e4m3))
    in_maps = []
    for c in range(NCORES):
        wv_sh = np.asarray(wv_w[c * VSH:(c + 1) * VSH, :], dtype=np.float32)
        in_maps.append({
            "X": X_dev,
            "X8": X8_dev,
            "WKQ": WKQ_dev,
            "WV": _swizzle(np.ascontiguousarray(wv_sh.T)),
            "KQB": KQB_dev,
            "VB": np.ascontiguousarray(np.broadcast_to(
                np.asarray(wv_b[c * VSH:(c + 1) * VSH], dtype=np.float32).reshape(1, VSH),
                (W, VSH))),
            "TRI": TRI,
            "SCL": SCL,
        })
    return in_maps


def run(inputs, trace=False):
    """Build + run on 8 cores; returns (full_output, BassKernelResults)."""
    nc = _build()
    in_maps = _make_in_maps(**inputs)
    res = run_bass_kernel_spmd(
        nc, in_maps, core_ids=list(range(NCORES)), trace=trace,
    )
    out = np.concatenate([res.results[c]["out"] for c in range(NCORES)], axis=1)
    return out, res


def kernel(**inputs):
    out, _ = run(inputs, trace=False)
    return out


if __name__ == "__main__":
    rng = np.random.default_rng(0)
    ins = {
        "x": rng.standard_normal((W, IN), dtype=np.float32),
        "wk_w": rng.standard_normal((KEY, IN), dtype=np.float32) / 90.5,
        "wk_b": rng.standard_normal((KEY,), dtype=np.float32) / 90.5,
        "wq_w": rng.standard_normal((KEY, IN), dtype=np.float32) / 90.5,
        "wq_b": rng.standard_normal((KEY,), dtype=np.float32) / 90.5,
        "wv_w": rng.standard_normal((VAL, IN), dtype=np.float32) / 90.5,
        "wv_b": rng.standard_normal((VAL,), dtype=np.float32) / 90.5,
        "tri": ((np.tril(np.full((W, W), 2.0, dtype=np.float32)) - 1.0) * 1e5),
    }
    out = kernel(**ins)
    print("out", out.shape, out.dtype, np.abs(out).mean())
